# revision 15
# baseline (speedup 1.0000x reference)
"""LIF (leaky integrate-and-fire) spiking recurrence on 8 Trainium2 cores.

Full input x: [T*bs, C, H, W] = [256, 128, 32, 32] f32 with T=8, bs=32.
Recurrence over T only, elementwise elsewhere:
    u_t = TAU * u_{t-1} * (1 - (u_t-1 > VTH)) + x_t ;  o_t = (u_t > VTH)

Sharding: fully data-parallel over batch (bs=32 -> 4 per core), no collectives.

Each core views its per-timestep [4,128,32,32] slab as a [128, 4096] tile.
The kernel is HBM-bound on the 16 MiB input; the output (pure 0/1 spikes)
is bit-packed on device so stores are 1 MiB instead of 16 MiB:

  DVE : one fused custom op per step   u_t = x_t + TAU * u_{t-1} * (u_{t-1} <= VTH)
        (in-place over the x_t slab; bit-exact vs the f32 reference).
        At t=T-1 a second custom op emits the spike bit directly:
        o_7 = (x_7 + TAU * u_6 * (u_6 <= VTH)) > VTH, skipping u_7 and the
        scalar-engine pass on the critical tail.
  ACT : s_t = Sign(VTH - u_t) in bf16 for t < 7  (-1 = spike, +1 = not)
  PE  : psum += diag(-2^(t-1)) @ s_t  (t<7)  and  psum += diag(128) @ o_7
        => psum = sum_t 2^t*o_t - 63.5
  ACT : packed_bf16 = psum + 63.5     (integers 0..255, exact in bf16)
  DMA : store packed [128, 4096] bf16; host unpacks bits to the f32 output.

Loads are ramped with small DMAs at both ends: small first so compute can
start early, small last because a DMA only signals completion as a whole
and its final descriptors drain on a single SDMA engine (~27 GB/s) — a big
tail DMA would gate the last timestep chain for ~7 us.
"""

import numpy as np
import ml_dtypes

import concourse.tile as tile
from concourse import bacc, mybir
from concourse.bass_utils import run_bass_kernel_spmd

T = 8
BS = 32
C = 128
HW = 32 * 32
NCORES = 8
BSH = BS // NCORES          # 4 batch elements per core
P = 128                     # SBUF partitions
FREE = BSH * C * HW // P    # 4096 f32 per partition per timestep
VTH = 1.0
TAU = 0.5
F32 = mybir.dt.float32
BF16 = mybir.dt.bfloat16

_nc_cache = None
_ops_cache = None


def _register_ops():
    """Register two fused LIF custom DVE ops:
       LIF_STEP_ANT: out = Src1 + (Src0 * C0) * (Src0 <= One)    [u update]
       LIF_LAST_ANT: out = (Src1 + (Src0 * C0) * (Src0 <= One)) > One
    i.e. u_new = x + TAU*u*(u <= VTH), and the final-step spike bit."""
    global _ops_cache
    if _ops_cache is not None:
        return _ops_cache
    import concourse.dve_ops as dve_ops
    from concourse.dve_spec import Spec, Src0, Src1, C0, One, lower
    from concourse.dve_uop import DveOpSpec

    u_new = Src1 + (Src0 * C0) * (Src0 <= One)
    specs = {
        "LIF_STEP_ANT": Spec(
            body=u_new,
            reference=lambda in0, in1, c0, c1, c2: in1
            + (in0 * np.float32(c0)) * (in0 <= np.float32(1.0)),
        ),
        "LIF_LAST_ANT": Spec(
            body=u_new > One,
            reference=lambda in0, in1, c0, c1, c2: (
                in1 + (in0 * np.float32(c0)) * (in0 <= np.float32(1.0))
                > np.float32(1.0)
            ).astype(np.float32),
        ),
    }
    ops = {}
    by_name = {op.name: op for op in dve_ops.OPS}
    for name, spec in specs.items():
        if name in by_name:
            ops[name] = by_name[name]
            continue
        row = dve_ops._CUSTOM_DVE_ROW_BASE + len(dve_ops.OPS)
        uops_sha = {}
        for ver in ("v3", "v4"):
            try:
                s = DveOpSpec(
                    name=name, opcode=row, uops=lower(spec, ver=ver), rd1_en=True
                )
                uops_sha[ver] = s.sha(ver)
            except Exception:
                pass
        op = dve_ops.DveOp(name, spec, subdim=False, uops_sha=uops_sha)
        dve_ops.OPS.append(op)
        dve_ops._SUB_OPCODE_FOR_NAME[name] = row
        dve_ops.CUSTOM_DVE_SPECS[name] = spec
        ops[name] = op
    _ops_cache = ops
    return ops


# Column chunking per timestep: fine chunks at the pipeline head (t<=1, so
# the DVE chain starts as soon as the first x_1 load lands) and toward the
# tail (t>=5, so the end-of-kernel chain drains with minimal latency).
_CHUNKS = {
    0: [(0, 1024), (1024, 2048), (2048, 3072), (3072, 4096)],
    1: [(0, 1024), (1024, 2048), (2048, 3072), (3072, 4096)],
    5: [(0, 2048), (2048, 3072), (3072, 4096)],
    6: [(0, 1024), (1024, 2048), (2048, 3072), (3072, 4096)],
    7: [(0, 1024), (1024, 2048), (2048, 3072), (3072, 3584), (3584, 4096)],
}
_DEFAULT_CHUNKS = [(0, 2048), (2048, 4096)]

# Load schedule in columns (1024 cols = 0.5 MiB). Small DMAs at both ends:
# small first so the DVE recurrence starts early, small last because a DMA
# signals completion only as a whole; 1 MiB mid-stream so each timestep
# slab completes in halves (halves the cross-DMA dependency wait).
_LOADS = [(0, 1024), (1024, 2048), (2048, 3072), (3072, 4096),          # x0
          (4096, 5120), (5120, 6144), (6144, 8192),                     # x1
          (8192, 10240), (10240, 12288), (12288, 14336), (14336, 16384),
          (16384, 18432), (18432, 20480), (20480, 22528), (22528, 24576),
          (24576, 26624), (26624, 28672),                               # x6
          (28672, 30720), (30720, 31744), (31744, 32256), (32256, 32768)]


def _build():
    ops = _register_ops()
    lif_step, lif_last = ops["LIF_STEP_ANT"], ops["LIF_LAST_ANT"]
    nc = bacc.Bacc("TRN2", target_bir_lowering=False, debug=False, num_devices=NCORES)
    x_d = nc.dram_tensor("x", [T, P, FREE], F32, kind="ExternalInput").ap()
    w_d = nc.dram_tensor("w", [P, T * 128], BF16, kind="ExternalInput").ap()
    o_d = nc.dram_tensor("o_pk", [P, FREE], BF16, kind="ExternalOutput").ap()

    with tile.TileContext(nc) as tc:
        with (
            tc.tile_pool(name="xa", bufs=1) as xa,
            tc.tile_pool(name="wp", bufs=1) as wp,
            tc.tile_pool(name="sp", bufs=3) as sp,
            tc.tile_pool(name="pk", bufs=1) as pkp,
            tc.tile_pool(name="cb", bufs=1) as cb,
            tc.tile_pool(name="ps", bufs=1, space="PSUM") as ps,
        ):
            # Whole 16 MiB per-core input resident in SBUF (128 KiB/partition);
            # u_t is computed in place over the x_t slab. Subtile dependency
            # tracking lets each compute chunk start once its load lands.
            xt = xa.tile([P, T * FREE], F32)
            xv = x_d.rearrange("t p f -> p t f")  # [128, T, FREE] HBM view

            # All DMA goes through HWDGE rings (sync/scalar): keeping SWDGE
            # fully idle avoids its descriptor-ring SBUF port contention,
            # which slows SDMA engine 15 and makes it straggle ~7 us.
            wsb = wp.tile([P, T * 128], BF16)     # 8 stationary diag matrices
            nc.sync.dma_start(out=wsb[:, :], in_=w_d)

            bias = cb.tile([P, 1], F32)
            nc.vector.memset(bias[:, :], 63.5)

            for a, b in _LOADS:
                t0, f0 = divmod(a, FREE)
                t1, f1 = divmod(b, FREE)
                if f0 == 0 and f1 == 0:
                    src = xv[:, t0:t1, :]
                else:
                    assert t1 == t0 or (t1 == t0 + 1 and f1 == 0)
                    src = xv[:, t0, f0:f1 if f1 else FREE]
                nc.sync.dma_start(out=xt[:, a:b], in_=src)

            psum = ps.tile([P, FREE], F32)        # packed-spike accumulator
            pk = pkp.tile([P, FREE], BF16)

            for t in range(T):
                last = t == T - 1
                s = sp.tile([P, FREE], BF16, name="s", tag="s")
                for a, b in _CHUNKS.get(t, _DEFAULT_CHUNKS):
                    xsl = xt[:, t * FREE + a:t * FREE + b]
                    if last:
                        # Fused u-update + threshold: o_7 directly in bf16.
                        nc.vector._custom_dve(
                            lif_last, out=s[:, a:b],
                            in0=xt[:, (t - 1) * FREE + a:(t - 1) * FREE + b],
                            in1=xsl, s0=TAU,
                        )
                    else:
                        if t > 0:
                            nc.vector._custom_dve(
                                lif_step, out=xsl,
                                in0=xt[:, (t - 1) * FREE + a:(t - 1) * FREE + b],
                                in1=xsl, s0=TAU,
                            )
                        # s = sign(VTH - u) in bf16: -1 = spike, +1 = not.
                        nc.scalar.activation(
                            s[:, a:b], xsl, mybir.ActivationFunctionType.Sign,
                            bias=VTH, scale=-1.0,
                        )
                    # psum += diag(-2^(t-1)) @ s_t (t<7); diag(128) @ o_7 (t=7)
                    for blk in range(a, b, 512):
                        nc.tensor.matmul(
                            psum[:, blk:blk + 512],
                            wsb[:, t * 128:(t + 1) * 128],
                            s[:, blk:blk + 512],
                            start=(t == 0),
                            stop=last,
                        )
                    if last:
                        # Convert psum -> packed bytes (0..255 ints, exact in
                        # bf16) and store, per chunk, right behind the PE.
                        nc.scalar.activation(
                            pk[:, a:b], psum[:, a:b],
                            mybir.ActivationFunctionType.Identity,
                            bias=bias[:, :], scale=1.0,
                        )
                        nc.scalar.dma_start(out=o_d[:, a:b], in_=pk[:, a:b])

    nc.compile()
    return nc


def _get_nc():
    global _nc_cache
    if _nc_cache is None:
        _nc_cache = _build()
    return _nc_cache


def _make_w():
    w = np.zeros((T, 128, 128), np.float32)
    for t in range(T - 1):
        np.fill_diagonal(w[t], -(2.0 ** (t - 1)))
    np.fill_diagonal(w[T - 1], 128.0)
    # SBUF layout: [partition k, t, m] -> [128, T*128]
    return np.ascontiguousarray(w.transpose(1, 0, 2)).reshape(P, T * 128).astype(
        ml_dtypes.bfloat16
    )


def _run(x: np.ndarray, **spmd_kwargs):
    nc = _get_nc()
    xr = np.ascontiguousarray(np.asarray(x, dtype=np.float32)).reshape(T, BS, C, HW)
    wl = _make_w()
    in_maps = [
        {
            "x": np.ascontiguousarray(xr[:, k * BSH:(k + 1) * BSH]).reshape(T, P, FREE),
            "w": wl,
        }
        for k in range(NCORES)
    ]
    res = run_bass_kernel_spmd(nc, in_maps, core_ids=list(range(NCORES)), **spmd_kwargs)
    out = np.empty((T, BS, C, HW), dtype=np.float32)
    for k in range(NCORES):
        pk = np.asarray(res.results[k]["o_pk"], dtype=np.float32)  # [P, FREE]
        b = pk.astype(np.uint8).reshape(-1)                        # exact ints
        bits = np.unpackbits(b[:, None], axis=1, bitorder="little")[:, :T]
        o = bits.T.astype(np.float32).reshape(T, BSH, C, HW)
        out[:, k * BSH:(k + 1) * BSH] = o
    return out.reshape(T * BS, C, 32, 32), res


def kernel(x: np.ndarray) -> np.ndarray:
    out, _ = _run(x)
    return out


# revision 17
# speedup vs baseline: 1.1325x; 1.1325x over previous
"""LIF (leaky integrate-and-fire) spiking recurrence on 8 Trainium2 cores.

Full input x: [T*bs, C, H, W] = [256, 128, 32, 32] f32 with T=8, bs=32.
Recurrence over T only, elementwise elsewhere:
    u_t = TAU * u_{t-1} * (1 - (u_t-1 > VTH)) + x_t ;  o_t = (u_t > VTH)

Sharding: fully data-parallel over batch (bs=32 -> 4 per core), no collectives.

Each core views its per-timestep [4,128,32,32] slab as a [128, 4096] tile.
The kernel is HBM-bound on the 16 MiB input; the output (pure 0/1 spikes)
is bit-packed on device so stores are 1 MiB instead of 16 MiB:

  DVE : one fused custom op per step   u_t = x_t + TAU * u_{t-1} * (u_{t-1} <= VTH)
        (in-place over the x_t slab; bit-exact vs the f32 reference).
        At t=T-1 a second custom op emits the spike bit directly:
        o_7 = (x_7 + TAU * u_6 * (u_6 <= VTH)) > VTH, skipping u_7 and the
        scalar-engine pass on the critical tail.
  ACT : s_t = Sign(VTH - u_t) in bf16 for t < 7  (-1 = spike, +1 = not)
  PE  : psum += diag(-2^(t-1)) @ s_t  (t<7)  and  psum += diag(128) @ o_7
        => psum = sum_t 2^t*o_t - 63.5
  ACT : packed_bf16 = psum + 63.5     (integers 0..255, exact in bf16)
  DMA : store packed [128, 4096] bf16; host unpacks bits to the f32 output.

Loads are ramped with small DMAs at both ends: small first so compute can
start early, small last because a DMA only signals completion as a whole
and its final descriptors drain on a single SDMA engine (~27 GB/s) — a big
tail DMA would gate the last timestep chain for ~7 us.
"""

import numpy as np
import ml_dtypes

import concourse.tile as tile
from concourse import bacc, mybir
from concourse.bass_utils import run_bass_kernel_spmd

T = 8
BS = 32
C = 128
HW = 32 * 32
NCORES = 8
BSH = BS // NCORES          # 4 batch elements per core
P = 128                     # SBUF partitions
FREE = BSH * C * HW // P    # 4096 f32 per partition per timestep
VTH = 1.0
TAU = 0.5
F32 = mybir.dt.float32
BF16 = mybir.dt.bfloat16

_nc_cache = None
_ops_cache = None


def _register_ops():
    """Register two fused LIF custom DVE ops:
       LIF_STEP_ANT: out = Src1 + (Src0 * C0) * (Src0 <= One)    [u update]
       LIF_LAST_ANT: out = (Src1 + (Src0 * C0) * (Src0 <= One)) > One
    i.e. u_new = x + TAU*u*(u <= VTH), and the final-step spike bit."""
    global _ops_cache
    if _ops_cache is not None:
        return _ops_cache
    import concourse.dve_ops as dve_ops
    from concourse.dve_spec import Spec, Src0, Src1, C0, One, lower
    from concourse.dve_uop import DveOpSpec

    u_new = Src1 + (Src0 * C0) * (Src0 <= One)
    specs = {
        "LIF_STEP_ANT": Spec(
            body=u_new,
            reference=lambda in0, in1, c0, c1, c2: in1
            + (in0 * np.float32(c0)) * (in0 <= np.float32(1.0)),
        ),
        "LIF_LAST_ANT": Spec(
            body=u_new > One,
            reference=lambda in0, in1, c0, c1, c2: (
                in1 + (in0 * np.float32(c0)) * (in0 <= np.float32(1.0))
                > np.float32(1.0)
            ).astype(np.float32),
        ),
    }
    ops = {}
    by_name = {op.name: op for op in dve_ops.OPS}
    for name, spec in specs.items():
        if name in by_name:
            ops[name] = by_name[name]
            continue
        row = dve_ops._CUSTOM_DVE_ROW_BASE + len(dve_ops.OPS)
        uops_sha = {}
        for ver in ("v3", "v4"):
            try:
                s = DveOpSpec(
                    name=name, opcode=row, uops=lower(spec, ver=ver), rd1_en=True
                )
                uops_sha[ver] = s.sha(ver)
            except Exception:
                pass
        op = dve_ops.DveOp(name, spec, subdim=False, uops_sha=uops_sha)
        dve_ops.OPS.append(op)
        dve_ops._SUB_OPCODE_FOR_NAME[name] = row
        dve_ops.CUSTOM_DVE_SPECS[name] = spec
        ops[name] = op
    _ops_cache = ops
    return ops


# Column chunking per timestep: fine chunks at the pipeline head (t<=1, so
# the DVE chain starts as soon as the first x_1 load lands) and toward the
# tail (t>=5, so the end-of-kernel chain drains with minimal latency).
_CHUNKS = {
    0: [(0, 1024), (1024, 2048), (2048, 3072), (3072, 4096)],
    1: [(0, 1024), (1024, 2048), (2048, 4096)],
    5: [(0, 2048), (2048, 3072), (3072, 4096)],
    6: [(0, 1024), (1024, 2048), (2048, 3072), (3072, 4096)],
    7: [(0, 1024), (1024, 2048), (2048, 3072), (3072, 3584), (3584, 4096)],
}
_DEFAULT_CHUNKS = [(0, 2048), (2048, 4096)]

# Load schedule in columns (1024 cols = 0.5 MiB). Small DMAs at both ends:
# small first so the DVE recurrence starts early, small last because a DMA
# signals completion only as a whole; 1 MiB mid-stream so each timestep
# slab completes in halves (halves the cross-DMA dependency wait).
_LOADS = [(0, 1024), (1024, 2048), (2048, 4096),                        # x0
          (4096, 5120), (5120, 6144), (6144, 8192),                     # x1
          (8192, 12288), (12288, 16384), (16384, 20480), (20480, 24576),
          (24576, 26624), (26624, 28672),                               # x6
          (28672, 30720), (30720, 31744), (31744, 32256), (32256, 32768)]


def _build():
    ops = _register_ops()
    lif_step, lif_last = ops["LIF_STEP_ANT"], ops["LIF_LAST_ANT"]
    nc = bacc.Bacc("TRN2", target_bir_lowering=False, debug=False, num_devices=NCORES)
    x_d = nc.dram_tensor("x", [T, P, FREE], F32, kind="ExternalInput").ap()
    w_d = nc.dram_tensor("w", [P, T * 128], BF16, kind="ExternalInput").ap()
    o_d = nc.dram_tensor("o_pk", [P, FREE], BF16, kind="ExternalOutput").ap()

    with tile.TileContext(nc) as tc:
        with (
            tc.tile_pool(name="xa", bufs=1) as xa,
            tc.tile_pool(name="wp", bufs=1) as wp,
            tc.tile_pool(name="sp", bufs=3) as sp,
            tc.tile_pool(name="pk", bufs=1) as pkp,
            tc.tile_pool(name="cb", bufs=1) as cb,
            tc.tile_pool(name="ps", bufs=1, space="PSUM") as ps,
        ):
            # Whole 16 MiB per-core input resident in SBUF (128 KiB/partition);
            # u_t is computed in place over the x_t slab. Subtile dependency
            # tracking lets each compute chunk start once its load lands.
            xt = xa.tile([P, T * FREE], F32)
            xv = x_d.rearrange("t p f -> p t f")  # [128, T, FREE] HBM view

            # All DMA goes through HWDGE rings (sync/scalar): keeping SWDGE
            # fully idle avoids its descriptor-ring SBUF port contention,
            # which slows SDMA engine 15 and makes it straggle ~7 us.
            wsb = wp.tile([P, T * 128], BF16)     # 8 stationary diag matrices
            nc.sync.dma_start(out=wsb[:, :], in_=w_d)

            bias = cb.tile([P, 1], F32)
            nc.vector.memset(bias[:, :], 63.5)

            for a, b in _LOADS:
                t0, f0 = divmod(a, FREE)
                t1, f1 = divmod(b, FREE)
                if f0 == 0 and f1 == 0:
                    src = xv[:, t0:t1, :]
                else:
                    assert t1 == t0 or (t1 == t0 + 1 and f1 == 0)
                    src = xv[:, t0, f0:f1 if f1 else FREE]
                nc.sync.dma_start(out=xt[:, a:b], in_=src)

            psum = ps.tile([P, FREE], F32)        # packed-spike accumulator
            pk = pkp.tile([P, FREE], BF16)

            for t in range(T):
                last = t == T - 1
                s = sp.tile([P, FREE], BF16, name="s", tag="s")
                for a, b in _CHUNKS.get(t, _DEFAULT_CHUNKS):
                    xsl = xt[:, t * FREE + a:t * FREE + b]
                    if last:
                        # Fused u-update + threshold: o_7 directly in bf16.
                        nc.vector._custom_dve(
                            lif_last, out=s[:, a:b],
                            in0=xt[:, (t - 1) * FREE + a:(t - 1) * FREE + b],
                            in1=xsl, s0=TAU,
                        )
                    else:
                        if t > 0:
                            nc.vector._custom_dve(
                                lif_step, out=xsl,
                                in0=xt[:, (t - 1) * FREE + a:(t - 1) * FREE + b],
                                in1=xsl, s0=TAU,
                            )
                        # s = sign(VTH - u) in bf16: -1 = spike, +1 = not.
                        nc.scalar.activation(
                            s[:, a:b], xsl, mybir.ActivationFunctionType.Sign,
                            bias=VTH, scale=-1.0,
                        )
                    # psum += diag(-2^(t-1)) @ s_t (t<7); diag(128) @ o_7 (t=7)
                    for blk in range(a, b, 512):
                        nc.tensor.matmul(
                            psum[:, blk:blk + 512],
                            wsb[:, t * 128:(t + 1) * 128],
                            s[:, blk:blk + 512],
                            start=(t == 0),
                            stop=last,
                        )
                    if last:
                        # Convert psum -> packed bytes (0..255 ints, exact in
                        # bf16) and store, per chunk, right behind the PE.
                        nc.scalar.activation(
                            pk[:, a:b], psum[:, a:b],
                            mybir.ActivationFunctionType.Identity,
                            bias=bias[:, :], scale=1.0,
                        )
                        nc.scalar.dma_start(out=o_d[:, a:b], in_=pk[:, a:b])

    nc.compile()
    return nc


def _get_nc():
    global _nc_cache
    if _nc_cache is None:
        _nc_cache = _build()
    return _nc_cache


def _make_w():
    w = np.zeros((T, 128, 128), np.float32)
    for t in range(T - 1):
        np.fill_diagonal(w[t], -(2.0 ** (t - 1)))
    np.fill_diagonal(w[T - 1], 128.0)
    # SBUF layout: [partition k, t, m] -> [128, T*128]
    return np.ascontiguousarray(w.transpose(1, 0, 2)).reshape(P, T * 128).astype(
        ml_dtypes.bfloat16
    )


def _run(x: np.ndarray, **spmd_kwargs):
    nc = _get_nc()
    xr = np.ascontiguousarray(np.asarray(x, dtype=np.float32)).reshape(T, BS, C, HW)
    wl = _make_w()
    in_maps = [
        {
            "x": np.ascontiguousarray(xr[:, k * BSH:(k + 1) * BSH]).reshape(T, P, FREE),
            "w": wl,
        }
        for k in range(NCORES)
    ]
    res = run_bass_kernel_spmd(nc, in_maps, core_ids=list(range(NCORES)), **spmd_kwargs)
    out = np.empty((T, BS, C, HW), dtype=np.float32)
    for k in range(NCORES):
        pk = np.asarray(res.results[k]["o_pk"], dtype=np.float32)  # [P, FREE]
        b = pk.astype(np.uint8).reshape(-1)                        # exact ints
        bits = np.unpackbits(b[:, None], axis=1, bitorder="little")[:, :T]
        o = bits.T.astype(np.float32).reshape(T, BSH, C, HW)
        out[:, k * BSH:(k + 1) * BSH] = o
    return out.reshape(T * BS, C, 32, 32), res


def kernel(x: np.ndarray) -> np.ndarray:
    out, _ = _run(x)
    return out


# revision 20
# speedup vs baseline: 1.1534x; 1.0184x over previous
"""LIF (leaky integrate-and-fire) spiking recurrence on 8 Trainium2 cores.

Full input x: [T*bs, C, H, W] = [256, 128, 32, 32] f32 with T=8, bs=32.
Recurrence over T only, elementwise elsewhere:
    u_t = TAU * u_{t-1} * (1 - (u_t-1 > VTH)) + x_t ;  o_t = (u_t > VTH)

Sharding: fully data-parallel over batch (bs=32 -> 4 per core), no collectives.

Each core views its per-timestep [4,128,32,32] slab as a [128, 4096] tile.
The kernel is HBM-bound on the 16 MiB input; the output (pure 0/1 spikes)
is bit-packed on device so stores are 1 MiB instead of 16 MiB:

  DVE : one fused custom op per step   u_t = x_t + TAU * u_{t-1} * (u_{t-1} <= VTH)
        (in-place over the x_t slab; bit-exact vs the f32 reference).
        At t=T-1 a second custom op emits the spike bit directly:
        o_7 = (x_7 + TAU * u_6 * (u_6 <= VTH)) > VTH, skipping u_7 and the
        scalar-engine pass on the critical tail.
  ACT : s_t = Sign(VTH - u_t) in bf16 for t < 7  (-1 = spike, +1 = not)
  PE  : psum += diag(-2^(t-1)) @ s_t  (t<7)  and  psum += diag(128) @ o_7
        => psum = sum_t 2^t*o_t - 63.5
  ACT : packed_bf16 = psum + 63.5     (integers 0..255, exact in bf16)
  DMA : store packed [128, 4096] bf16; host unpacks bits to the f32 output.

Loads are ramped with small DMAs at both ends: small first so compute can
start early, small last because a DMA only signals completion as a whole
and its final descriptors drain on a single SDMA engine (~27 GB/s) — a big
tail DMA would gate the last timestep chain for ~7 us.
"""

import numpy as np
import ml_dtypes

import concourse.tile as tile
from concourse import bacc, mybir
from concourse.bass_utils import run_bass_kernel_spmd

T = 8
BS = 32
C = 128
HW = 32 * 32
NCORES = 8
BSH = BS // NCORES          # 4 batch elements per core
P = 128                     # SBUF partitions
FREE = BSH * C * HW // P    # 4096 f32 per partition per timestep
VTH = 1.0
TAU = 0.5
F32 = mybir.dt.float32
BF16 = mybir.dt.bfloat16

_nc_cache = None
_ops_cache = None


def _register_ops():
    """Register two fused LIF custom DVE ops:
       LIF_STEP_ANT: out = Src1 + (Src0 * C0) * (Src0 <= One)    [u update]
       LIF_LAST_ANT: out = (Src1 + (Src0 * C0) * (Src0 <= One)) > One
    i.e. u_new = x + TAU*u*(u <= VTH), and the final-step spike bit."""
    global _ops_cache
    if _ops_cache is not None:
        return _ops_cache
    import concourse.dve_ops as dve_ops
    from concourse.dve_spec import Spec, Src0, Src1, C0, One, lower
    from concourse.dve_uop import DveOpSpec

    u_new = Src1 + (Src0 * C0) * (Src0 <= One)
    specs = {
        "LIF_STEP_ANT": Spec(
            body=u_new,
            reference=lambda in0, in1, c0, c1, c2: in1
            + (in0 * np.float32(c0)) * (in0 <= np.float32(1.0)),
        ),
        "LIF_LAST_ANT": Spec(
            body=u_new > One,
            reference=lambda in0, in1, c0, c1, c2: (
                in1 + (in0 * np.float32(c0)) * (in0 <= np.float32(1.0))
                > np.float32(1.0)
            ).astype(np.float32),
        ),
    }
    ops = {}
    by_name = {op.name: op for op in dve_ops.OPS}
    for name, spec in specs.items():
        if name in by_name:
            ops[name] = by_name[name]
            continue
        row = dve_ops._CUSTOM_DVE_ROW_BASE + len(dve_ops.OPS)
        uops_sha = {}
        for ver in ("v3", "v4"):
            try:
                s = DveOpSpec(
                    name=name, opcode=row, uops=lower(spec, ver=ver), rd1_en=True
                )
                uops_sha[ver] = s.sha(ver)
            except Exception:
                pass
        op = dve_ops.DveOp(name, spec, subdim=False, uops_sha=uops_sha)
        dve_ops.OPS.append(op)
        dve_ops._SUB_OPCODE_FOR_NAME[name] = row
        dve_ops.CUSTOM_DVE_SPECS[name] = spec
        ops[name] = op
    _ops_cache = ops
    return ops


# Column chunking per timestep: fine chunks at the pipeline head (t<=1, so
# the DVE chain starts as soon as the first x_1 load lands) and toward the
# tail (t>=5, so the end-of-kernel chain drains with minimal latency).
_CHUNKS = {
    0: [(0, 1024), (1024, 2048), (2048, 3072), (3072, 4096)],
    1: [(0, 1024), (1024, 2048), (2048, 4096)],
    5: [(0, 2048), (2048, 4096)],
    6: [(0, 2048), (2048, 4096)],
    7: [(0, 1024), (1024, 2048), (2048, 3072), (3072, 3584), (3584, 4096)],
}
_DEFAULT_CHUNKS = [(0, 2048), (2048, 4096)]

# Load schedule in columns (1024 cols = 0.5 MiB). Small DMAs at both ends:
# small first so the DVE recurrence starts early, small last because a DMA
# signals completion only as a whole; 1 MiB mid-stream so each timestep
# slab completes in halves (halves the cross-DMA dependency wait).
_LOADS = [(0, 1024), (1024, 2048), (2048, 4096),                        # x0
          (4096, 5120), (5120, 6144), (6144, 8192),                     # x1
          (8192, 12288), (12288, 16384),
          (16384, 18432), (18432, 20480), (20480, 22528), (22528, 24576),
          (24576, 26624), (26624, 28672),                               # x6
          (28672, 30720), (30720, 31744), (31744, 32256), (32256, 32768)]


def _build():
    ops = _register_ops()
    lif_step, lif_last = ops["LIF_STEP_ANT"], ops["LIF_LAST_ANT"]
    nc = bacc.Bacc("TRN2", target_bir_lowering=False, debug=False, num_devices=NCORES)
    x_d = nc.dram_tensor("x", [T, P, FREE], F32, kind="ExternalInput").ap()
    w_d = nc.dram_tensor("w", [P, T * 128], BF16, kind="ExternalInput").ap()
    o_d = nc.dram_tensor("o_pk", [P, FREE], BF16, kind="ExternalOutput").ap()

    with tile.TileContext(nc) as tc:
        with (
            tc.tile_pool(name="xa", bufs=1) as xa,
            tc.tile_pool(name="wp", bufs=1) as wp,
            tc.tile_pool(name="sp", bufs=3) as sp,
            tc.tile_pool(name="pk", bufs=1) as pkp,
            tc.tile_pool(name="cb", bufs=1) as cb,
            tc.tile_pool(name="ps", bufs=1, space="PSUM") as ps,
        ):
            # Whole 16 MiB per-core input resident in SBUF (128 KiB/partition);
            # u_t is computed in place over the x_t slab. Subtile dependency
            # tracking lets each compute chunk start once its load lands.
            xt = xa.tile([P, T * FREE], F32)
            xv = x_d.rearrange("t p f -> p t f")  # [128, T, FREE] HBM view

            # All DMA goes through HWDGE rings (sync/scalar): keeping SWDGE
            # fully idle avoids its descriptor-ring SBUF port contention,
            # which slows SDMA engine 15 and makes it straggle ~7 us.
            wsb = wp.tile([P, T * 128], BF16)     # 8 stationary diag matrices
            nc.sync.dma_start(out=wsb[:, :], in_=w_d)

            bias = cb.tile([P, 1], F32)
            nc.vector.memset(bias[:, :], 63.5)

            for a, b in _LOADS:
                t0, f0 = divmod(a, FREE)
                t1, f1 = divmod(b, FREE)
                if f0 == 0 and f1 == 0:
                    src = xv[:, t0:t1, :]
                else:
                    assert t1 == t0 or (t1 == t0 + 1 and f1 == 0)
                    src = xv[:, t0, f0:f1 if f1 else FREE]
                nc.sync.dma_start(out=xt[:, a:b], in_=src)

            psum = ps.tile([P, FREE], F32)        # packed-spike accumulator
            pk = pkp.tile([P, FREE], BF16)

            for t in range(T):
                last = t == T - 1
                s = sp.tile([P, FREE], BF16, name="s", tag="s")
                for a, b in _CHUNKS.get(t, _DEFAULT_CHUNKS):
                    xsl = xt[:, t * FREE + a:t * FREE + b]
                    if last:
                        # Fused u-update + threshold: o_7 directly in bf16.
                        nc.vector._custom_dve(
                            lif_last, out=s[:, a:b],
                            in0=xt[:, (t - 1) * FREE + a:(t - 1) * FREE + b],
                            in1=xsl, s0=TAU,
                        )
                    else:
                        if t > 0:
                            nc.vector._custom_dve(
                                lif_step, out=xsl,
                                in0=xt[:, (t - 1) * FREE + a:(t - 1) * FREE + b],
                                in1=xsl, s0=TAU,
                            )
                        # s = sign(VTH - u) in bf16: -1 = spike, +1 = not.
                        nc.scalar.activation(
                            s[:, a:b], xsl, mybir.ActivationFunctionType.Sign,
                            bias=VTH, scale=-1.0,
                        )
                    # psum += diag(-2^(t-1)) @ s_t (t<7); diag(128) @ o_7 (t=7)
                    for blk in range(a, b, 512):
                        nc.tensor.matmul(
                            psum[:, blk:blk + 512],
                            wsb[:, t * 128:(t + 1) * 128],
                            s[:, blk:blk + 512],
                            start=(t == 0),
                            stop=last,
                        )
                    if last:
                        # Convert psum -> packed bytes (0..255 ints, exact in
                        # bf16) and store, per chunk, right behind the PE.
                        # The last two chunks convert on the vector engine,
                        # which is free by then, while ACT finishes earlier
                        # chunks; stores issue from gpsimd (SWDGE) so they
                        # don't occupy a compute engine's queue.
                        if a >= 3072:
                            nc.vector.tensor_scalar(
                                pk[:, a:b], psum[:, a:b], 63.5, None,
                                mybir.AluOpType.add,
                            )
                        else:
                            nc.scalar.activation(
                                pk[:, a:b], psum[:, a:b],
                                mybir.ActivationFunctionType.Identity,
                                bias=bias[:, :], scale=1.0,
                            )
                        nc.gpsimd.dma_start(out=o_d[:, a:b], in_=pk[:, a:b])

    nc.compile()
    return nc


def _get_nc():
    global _nc_cache
    if _nc_cache is None:
        _nc_cache = _build()
    return _nc_cache


def _make_w():
    w = np.zeros((T, 128, 128), np.float32)
    for t in range(T - 1):
        np.fill_diagonal(w[t], -(2.0 ** (t - 1)))
    np.fill_diagonal(w[T - 1], 128.0)
    # SBUF layout: [partition k, t, m] -> [128, T*128]
    return np.ascontiguousarray(w.transpose(1, 0, 2)).reshape(P, T * 128).astype(
        ml_dtypes.bfloat16
    )


def _run(x: np.ndarray, **spmd_kwargs):
    nc = _get_nc()
    xr = np.ascontiguousarray(np.asarray(x, dtype=np.float32)).reshape(T, BS, C, HW)
    wl = _make_w()
    in_maps = [
        {
            "x": np.ascontiguousarray(xr[:, k * BSH:(k + 1) * BSH]).reshape(T, P, FREE),
            "w": wl,
        }
        for k in range(NCORES)
    ]
    res = run_bass_kernel_spmd(nc, in_maps, core_ids=list(range(NCORES)), **spmd_kwargs)
    out = np.empty((T, BS, C, HW), dtype=np.float32)
    for k in range(NCORES):
        pk = np.asarray(res.results[k]["o_pk"], dtype=np.float32)  # [P, FREE]
        b = pk.astype(np.uint8).reshape(-1)                        # exact ints
        bits = np.unpackbits(b[:, None], axis=1, bitorder="little")[:, :T]
        o = bits.T.astype(np.float32).reshape(T, BSH, C, HW)
        out[:, k * BSH:(k + 1) * BSH] = o
    return out.reshape(T * BS, C, 32, 32), res


def kernel(x: np.ndarray) -> np.ndarray:
    out, _ = _run(x)
    return out
